# revision 21
# baseline (speedup 1.0000x reference)
"""Trainium2 Bass kernel for nn_AttentionLayer (pre-conv + BN/ReLU, QK^T
softmax attention, V aggregation, residual, final conv + BN/ReLU).

Data-parallel over batch: 8 samples -> 8 NeuronCores, zero collectives.
Everything streams bf16 into fp32 psum; output DMA'd as bf16.

Software-pipelined single-stream schedule (one For_i body = one sample):
  - Stage A (3x3 conv 512->256, 36 shifted matmuls/row-block) interleaves
    the 1x1 q/k convs, the vT chunks, and i-block-0's energy pairs as soon
    as their a1/q/k columns complete, so the ScalarE exp chain starts ~60us
    before stage A ends. Head DMA pieces alternate between the sync/scalar
    rings in exact first-use order (shared serial HWDGE + DMA pool).
  - Stage C phases: energy pairs of i-block k interleave with the
    transposed-AV matmuls of i-block k-1 (ut double-buffered by phase
    parity), so the PE never idles on the ScalarE exp chain.
  - Transposed AV: out[c,i] = sum_j vT[j,c]*ut[j,i] -- the attention output
    lands directly in channel-major layout (no per-isub PE transposes).
    The softmax denominator comes from DVE chunk-sums of ut, a
    column-of-ones matmul per 128-i chunk, tiny PE transposes, and one
    gpsimd partition_broadcast; the normalize + gamma*b_v + residual are
    two DVE ops per channel half.
  - Stage D (final 3x3 conv) row-blocks interleave into phases 3+ and the
    tail; the last row-block is 2 rows so the final act->dma->barrier
    chain after the last matmul is short.
"""
import sys

sys.path.insert(0, "/opt/trn_rl_repo")

import numpy as np

import concourse.bass as bass  # noqa: F401
import concourse.mybir as mybir
import concourse.tile as tile
from concourse import bacc
from concourse.bass_utils import run_bass_kernel_spmd
from concourse.masks import make_identity

F32 = mybir.dt.float32
BF16 = mybir.dt.bfloat16

B, CIN, COUT, H, W = 8, 512, 256, 48, 48
HP, WP = H + 2, W + 2  # padded feature map
NP2 = HP * WP  # 2500
N = H * W  # 2304
CQK = 64
NJ = N // 128  # 18
ROWBLOCKS = [(0, 10), (10, 20), (20, 30), (30, 40), (40, 48)]
IBLOCKS = [(0, 512), (512, 1024), (1024, 1536), (1536, 2048), (2048, 2304)]
EXP_SHIFT = -30.0  # exp(E + shift): cancels in softmax, guards overflow
EPS = 1e-5

_NC_CACHE = {}

ADD = mybir.AluOpType.add


def build_nc(loop_reps=None, sim_bodies=1, ablate=()):
    """loop_reps: wrap the body in a device-side For_i loop (timing builds).
    sim_bodies: without loop_reps, emit the body N times back-to-back
    (steady-state analysis under the timeline simulator).
    ablate: timing-diagnostic switches ("empty", "no_dma", "no_a", "no_attn",
    "no_d"); default () = the real kernel."""
    nc = bacc.Bacc("TRN2")

    x_d = nc.declare_dram_parameter("x", [CIN, NP2], BF16, isOutput=False)
    wpre_d = nc.declare_dram_parameter("wpre", [128, 72, 128], BF16, isOutput=False)
    wqk_d = nc.declare_dram_parameter("wqk", [128, 2, 256], BF16, isOutput=False)
    wv_d = nc.declare_dram_parameter("wv", [128, 2, 256], BF16, isOutput=False)
    wf_d = nc.declare_dram_parameter("wf", [128, 36, 128], BF16, isOutput=False)
    const_d = nc.declare_dram_parameter("consts", [128, 12], F32, isOutput=False)
    out_d = nc.declare_dram_parameter("out", [COUT, N], BF16, isOutput=True)

    RELU = mybir.ActivationFunctionType.Relu
    IDENT = mybir.ActivationFunctionType.Identity
    EXP = mybir.ActivationFunctionType.Exp

    with (
        tile.TileContext(nc) as tc,
        tc.tile_pool(name="consts", bufs=1) as consts,
        tc.tile_pool(name="data", bufs=1) as data,
        tc.tile_pool(name="attp", bufs=6) as attp,
        tc.tile_pool(name="outp", bufs=4) as outp,
        tc.tile_pool(name="acc", bufs=4, space="PSUM") as acc,
        tc.tile_pool(name="epool", bufs=2, space="PSUM") as epool,
    ):
        x_sb = data.tile([128, 4, HP, WP], BF16, tag="x")
        wpre_sb = consts.tile([128, 72, 128], BF16, tag="wpre")
        const_sb = consts.tile([128, 12], F32, tag="const")
        wqk_sb = consts.tile([128, 2, 256], BF16, tag="wqk")
        wv_sb = consts.tile([128, 2, 256], BF16, tag="wv")
        wf_sb = consts.tile([128, 36, 128], BF16, tag="wf")
        a1_sb = data.tile([128, 2, N], BF16, tag="a1")
        # q/k duplicated across both partition halves: enables row-packed
        # K=64 energy matmuls (two j-chunks concurrently in the PE array)
        q_sb = data.tile([128, N], BF16, tag="q")
        k_sb = data.tile([128, N], BF16, tag="k")
        vt_sb = data.tile([128, NJ, 256], BF16, tag="vt")
        # ut double-buffered by i-block parity: exp of i-block k writes
        # parity k%2 while the AV of i-block k-1 reads parity (k-1)%2.
        ut_sb = data.tile([128, 2, NJ, 512], BF16, tag="ut")
        feat_sb = data.tile([128, 2, N], BF16, tag="feat")
        fpad_sb = data.tile([128, 2, HP, WP], BF16, tag="fpad")
        ebias_sb = consts.tile([128, 1], F32, tag="ebias")
        ident_sb = consts.tile([128, 128], BF16, tag="ident")
        onec_sb = consts.tile([128, 1], BF16, tag="onec")
        # softmax denominator: row-broadcast reciprocal used to normalize
        # the AV output
        recb_sb = data.tile([128, 4, 128], BF16, tag="recb")

        def emit_loads():
            """Head DMAs. The HWDGE descriptor unit round-robins the sync and
            scalar rings and all transfers serialize on a shared DMA pool, so
            pieces alternate between rings in exact first-use order: wpre[0:3]
            | x kc0 head | wpre[3:9] | wpre kc1 | x kc1 head | ... so conv
            row-block 0 starts ~3us in."""

            def dma_x_head(ring, kc):
                ring.dma_start(
                    out=x_sb[:, kc, 0:13, :].rearrange("p h w -> p (h w)"),
                    in_=x_d[kc * 128 : (kc + 1) * 128, 0 : 13 * WP],
                )

            def dma_wpre(ring, s0, s1):
                ring.dma_start(out=wpre_sb[:, s0:s1, :], in_=wpre_d[:, s0:s1, :])

            def dma_x_band(ring, r0, r1):
                # rows r0:r1 of all 4 kc chunks (JIT bands for conv row-blocks)
                ring.dma_start(
                    out=x_sb[:, :, r0:r1, :].rearrange("p c h w -> p c (h w)"),
                    in_=x_d.rearrange("(c p) n -> p c n", c=4)[
                        :, :, r0 * WP : r1 * WP
                    ],
                )

            dma_wpre(nc.sync, 0, 3)
            dma_x_head(nc.sync, 0)
            dma_wpre(nc.sync, 3, 9)
            dma_wpre(nc.sync, 9, 18)
            dma_x_head(nc.sync, 1)
            dma_wpre(nc.sync, 18, 27)
            dma_x_head(nc.sync, 2)
            dma_wpre(nc.sync, 27, 36)
            dma_x_head(nc.sync, 3)
            dma_wpre(nc.sync, 36, 45)
            dma_wpre(nc.sync, 45, 54)
            dma_wpre(nc.sync, 54, 63)
            dma_wpre(nc.sync, 63, 72)
            nc.sync.dma_start(out=const_sb[:], in_=const_d[:])
            # bulk of x (rows 13..49, all kc chunks) in one piece
            nc.sync.dma_start(
                out=x_sb[:, :, 13:, :].rearrange("p c h w -> p c (h w)"),
                in_=x_d.rearrange("(c p) n -> p c n", c=4)[:, :, 13 * WP :],
            )
            nc.sync.dma_start(out=wqk_sb[:], in_=wqk_d[:])
            nc.sync.dma_start(out=wv_sb[:], in_=wv_d[:])
            nc.sync.dma_start(out=wf_sb[:], in_=wf_d[:])

        # ---- one-time setup: constants that no iteration clobbers
        nc.vector.memset(ebias_sb[:], EXP_SHIFT)
        nc.vector.memset(onec_sb[:], 1.0)
        make_identity(nc, ident_sb[:])
        # fpad borders stay zero; row copies only touch the interior
        nc.vector.memset(fpad_sb[:, :, 0:1, :], 0.0)
        nc.vector.memset(fpad_sb[:, :, HP - 1 : HP, :], 0.0)
        nc.vector.memset(fpad_sb[:, :, 1 : 1 + H, 0:1], 0.0)
        nc.vector.memset(fpad_sb[:, :, 1 : 1 + H, WP - 1 : WP], 0.0)

        def emit_body():
            if "empty" in ablate:
                nc.vector.memset(recb_sb[:, 0:1, 0:4], 0.0)
                return
            if "only_dma" in ablate:
                emit_loads()
                return
            if "no_dma" not in ablate:
                emit_loads()
            else:
                # satisfy the tile allocator: every read tile needs a writer
                nc.vector.memset(x_sb[:, :, 0:1, 0:4], 0.0)
                nc.vector.memset(wpre_sb[:, 0:1, 0:4], 0.0)
                nc.vector.memset(wqk_sb[:, 0:1, 0:4], 0.0)
                nc.vector.memset(wv_sb[:, 0:1, 0:4], 0.0)
                nc.vector.memset(wf_sb[:, 0:1, 0:4], 0.0)
                nc.vector.memset(const_sb[:, 0:4], 0.0)
            if "no_a" in ablate:
                nc.vector.memset(a1_sb[:, :, 0:2], 0.0)
            if "no_attn" in ablate:
                nc.vector.memset(feat_sb[:, :, 0:2], 0.0)
            if "no_exp" in ablate or "no_energy" in ablate:
                nc.vector.memset(ut_sb[:, :, 0:1, 0:2], 0.01)
            if "no_den" in ablate:
                nc.vector.memset(recb_sb[:, :, 0:2], 1.0)

            # ---------------- emit helpers ----------------

            def conv3x3(ps, w_sb, slot_of, src4, kcs, h0, h1):
                taps = [
                    (kc, ty, tx)
                    for kc in range(kcs)
                    for ty in range(3)
                    for tx in range(3)
                ]
                for idx, (kc, ty, tx) in enumerate(taps):
                    nc.tensor.matmul(
                        ps[:, : (h1 - h0) * W],
                        lhsT=w_sb[:, slot_of(kc, ty, tx), :],
                        rhs=src4[:, kc, ty + h0 : ty + h1, tx : tx + W],
                        start=(idx == 0),
                        stop=(idx == len(taps) - 1),
                    )

            def emit_a_conv(h0, h1, m):
                wblk = (h1 - h0) * W
                ps = acc.tile([128, 512], F32, tag="acc")
                conv3x3(
                    ps, wpre_sb,
                    lambda kc, ty, tx, m=m: m * 36 + kc * 9 + ty * 3 + tx,
                    x_sb, 4, h0, h1,
                )
                nc.scalar.activation(
                    a1_sb[:, m, h0 * W : h1 * W], ps[:, :wblk], RELU,
                    scale=const_sb[:, m : m + 1],
                    bias=const_sb[:, 2 + m : 3 + m],
                )

            def emit_k(h0, h1):
                i0, i1 = h0 * W, h1 * W
                wi = i1 - i0
                psk = acc.tile([128, 512], F32, tag="acc")
                for kc in range(2):
                    nc.tensor.matmul(
                        psk[:, :wi],
                        lhsT=wqk_sb[:, kc, 128:256],
                        rhs=a1_sb[:, kc, i0:i1],
                        start=(kc == 0), stop=(kc == 1),
                    )
                nc.scalar.activation(
                    k_sb[:, i0:i1], psk[:, :wi], IDENT, bias=const_sb[:, 9:10]
                )

            def emit_q(h0, h1):
                i0, i1 = h0 * W, h1 * W
                wi = i1 - i0
                psq = acc.tile([128, 512], F32, tag="acc")
                for kc in range(2):
                    nc.tensor.matmul(
                        psq[:, :wi],
                        lhsT=wqk_sb[:, kc, 0:128],
                        rhs=a1_sb[:, kc, i0:i1],
                        start=(kc == 0), stop=(kc == 1),
                    )
                nc.scalar.activation(
                    q_sb[:, i0:i1], psq[:, :wi], IDENT, bias=const_sb[:, 8:9]
                )

            def emit_vt(j):
                psv = acc.tile([128, 512], F32, tag="acc")
                for kc in range(2):
                    nc.tensor.matmul(
                        psv[:, :256],
                        lhsT=a1_sb[:, kc, j * 128 : (j + 1) * 128],
                        rhs=wv_sb[:, kc, :],
                        start=(kc == 0), stop=(kc == 1),
                    )
                nc.vector.tensor_copy(vt_sb[:, j, 0:256], psv[:, :256])

            E_PAIR = 2

            def emit_epair(ib, jj):
                i0, i1 = IBLOCKS[ib]
                wi = i1 - i0
                par = ib % 2
                if "no_energy" in ablate:
                    return
                pse = epool.tile([128, E_PAIR, 512], F32, tag="e")
                for hh in range(E_PAIR):
                    j = E_PAIR * jj + hh
                    p0 = (hh % 2) * CQK  # alternate array row-halves
                    nc.tensor.matmul(
                        pse[:, hh, :wi],
                        lhsT=k_sb[p0 : p0 + CQK, j * 128 : (j + 1) * 128],
                        rhs=q_sb[p0 : p0 + CQK, i0:i1],
                        start=True, stop=True,
                    )
                if "no_exp" in ablate:
                    return
                nc.scalar.activation(
                    ut_sb[:, par, E_PAIR * jj : E_PAIR * (jj + 1), :wi],
                    pse[:, :, :wi], EXP,
                    bias=ebias_sb[:, 0:1],
                )

            # ---- transposed AV: out[c, i] = sum_j vt[j, c] * ut[j, i].
            # No PE transposes; the softmax denominator comes from DVE chunk
            # sums of ut + one column-of-ones matmul per 128-i chunk, and the
            # per-i reciprocal is row-broadcast via gpsimd partition_broadcast.

            def emit_avt_cc(ib, cc):
                i0, i1 = IBLOCKS[ib]
                wi = i1 - i0
                par = ib % 2
                pav = acc.tile([128, 512], F32, tag="acc")
                nj = 1 if "no_av" in ablate else NJ
                for j in range(nj):
                    nc.tensor.matmul(
                        pav[:, :wi],
                        lhsT=vt_sb[:, j, cc * 128 : (cc + 1) * 128],
                        rhs=ut_sb[:, par, j, 0:wi],
                        start=(j == 0), stop=(j == nj - 1),
                    )
                return pav

            def emit_denom(ib):
                # denominator on the PE: ones-matmul partition-sums of ut,
                # accumulated across the 18 j-chunks in f32 psum (the PE has
                # slack in the phase region; this kills the serial DVE
                # chunk-sum chain and its bf16 accumulation error).
                if "no_den" in ablate:
                    return
                wi = IBLOCKS[ib][1] - IBLOCKS[ib][0]
                par = ib % 2
                dnt = acc.tile([128, 512], F32, tag="acc")
                for j in range(NJ):
                    nc.tensor.matmul(
                        dnt[0:1, :wi],
                        lhsT=onec_sb[:, 0:1],
                        rhs=ut_sb[:, par, j, 0:wi],
                        start=(j == 0), stop=(j == NJ - 1),
                    )
                rbrow = attp.tile([1, 512], BF16, tag="rrow")
                with nc.allow_low_precision(reason="softmax 1/denom, gamma-diluted"):
                    nc.vector.reciprocal(rbrow[0:1, :wi], dnt[0:1, :wi])
                nc.gpsimd.partition_broadcast(
                    recb_sb[:].rearrange("p a b -> p (a b)")[:, 0:wi],
                    rbrow[0:1, 0:wi],
                )

            def emit_norm(ib, cc, pav):
                i0, i1 = IBLOCKS[ib]
                wi = i1 - i0
                u1 = attp.tile([128, 512], BF16, tag="att")
                nc.vector.tensor_mul(
                    u1[:, :wi], pav[:, :wi],
                    recb_sb[:].rearrange("p a b -> p (a b)")[:, :wi],
                )
                # feat = (att_norm + gamma*b_v) + a1
                nc.vector.scalar_tensor_tensor(
                    feat_sb[:, cc, i0:i1],
                    u1[:, :wi],
                    const_sb[:, 10 + cc : 11 + cc],
                    a1_sb[:, cc, i0:i1],
                    op0=ADD, op1=ADD,
                )

            def emit_fpad_rows(r0, r1):
                for cc in range(2):
                    nc.vector.tensor_copy(
                        fpad_sb[:, cc, 1 + r0 : 1 + r1, 1 : 1 + W],
                        feat_sb[:, cc, r0 * W : r1 * W].rearrange(
                            "p (h w) -> p h w", w=W
                        ),
                    )

            def emit_d_conv(h0, h1, m):
                ps = acc.tile([128, 512], F32, tag="acc")
                conv3x3(
                    ps, wf_sb,
                    lambda kc, ty, tx, m=m: m * 18 + kc * 9 + ty * 3 + tx,
                    fpad_sb, 2, h0, h1,
                )
                return ps

            def emit_d_act(ps, o2, h0, h1, m):
                wblk = (h1 - h0) * W
                nc.scalar.activation(
                    o2[:, m, :wblk], ps[:, :wblk], RELU,
                    scale=const_sb[:, 4 + m : 5 + m],
                    bias=const_sb[:, 6 + m : 7 + m],
                )

            def emit_d_out(o2, h0, h1, ring=None):
                wblk = (h1 - h0) * W
                (ring or nc.scalar).dma_start(
                    out=out_d.rearrange("(m p) n -> p m n", m=2)[
                        :, :, h0 * W : h1 * W
                    ],
                    in_=o2[:, :, :wblk],
                )

            # stage-D row-blocks: last block kept tiny so the final
            # act->dma->barrier tail after the last matmul is short.
            D_RB = [(0, 10), (10, 20), (20, 30), (30, 40), (40, 46), (46, 48)]

            def emit_d_rb(rb, between=(), ring=None):
                """Emit one stage-D row-block; `between` is a list of thunks
                interleaved at the two conv-half boundaries (keeps exp ahead
                of relu on the ScalarE queue)."""
                h0, h1 = D_RB[rb]
                its = list(between)
                o2 = outp.tile([128, 2, 480], BF16, tag="o")
                for m in range(2):
                    ps = emit_d_conv(h0, h1, m)
                    if its:
                        its.pop(0)()
                    emit_d_act(ps, o2, h0, h1, m)
                for t in its:
                    t()
                emit_d_out(o2, h0, h1, ring)

            # ---------------- schedule ----------------

            if ablate:
                # diagnostic schedules only (default path below is untouched)
                if "no_a" not in ablate:
                    for r, (h0, h1) in enumerate(ROWBLOCKS):
                        emit_a_conv(h0, h1, 0)
                        emit_a_conv(h0, h1, 1)
                if "no_attn" not in ablate:
                    for rb in range(5):
                        emit_k(*ROWBLOCKS[rb])
                        emit_q(*ROWBLOCKS[rb])
                    for j in range(NJ):
                        emit_vt(j)
                    for p in range(5):
                        ib = p
                        for jj in range(9):
                            emit_epair(ib, jj)
                        pav0 = emit_avt_cc(ib, 0)
                        pav1 = emit_avt_cc(ib, 1)
                        emit_denom(ib)
                        emit_norm(ib, 0, pav0)
                        emit_norm(ib, 1, pav1)
                if "no_d" not in ablate:
                    emit_fpad_rows(0, 48)
                    for rb in range(len(D_RB)):
                        emit_d_rb(rb)
                else:
                    nc.scalar.dma_start(
                        out=out_d.rearrange("(m p) n -> p m n", m=2),
                        in_=feat_sb[:, :, :],
                    )
                return

            # Stage A/B: conv row-blocks; row-block r's vT/q/k/i-block-0
            # energy pairs are deferred until after conv(r+1, m0) so their
            # psum-bank WARs and a1-activation dependencies are long settled.
            A_TAIL = {
                0: [],
                1: [("e", (0, 3))],
                2: [("e", (3, 5))],
                3: [("e", (5, 7))],
            }
            VT_OF_RB = {0: (0, 3), 1: (3, 7), 2: (7, 11), 3: (11, 15), 4: (15, 18)}
            for r, (h0, h1) in enumerate(ROWBLOCKS):
                emit_a_conv(h0, h1, 0)
                emit_a_conv(h0, h1, 1)
                if r > 0:
                    pr = r - 1
                    ph0, ph1 = ROWBLOCKS[pr]
                    for j in range(*VT_OF_RB[pr]):
                        emit_vt(j)
                    emit_k(ph0, ph1)
                    emit_q(ph0, ph1)
                    for kind, (a, b) in A_TAIL[pr]:
                        for idx in range(a, b):
                            emit_epair(0, idx)
            # row-block 4 tail: k/q/vT acts first so the two big i-block-0
            # exps don't block their psum drains on the ScalarE queue; the
            # energy pairs space the last vT copies.
            emit_k(*ROWBLOCKS[4])
            emit_q(*ROWBLOCKS[4])
            emit_vt(15)
            emit_epair(0, 7)
            emit_vt(16)
            emit_epair(0, 8)
            emit_vt(17)

            # fpad rows fully determined by each i-block's feat columns
            FPAD_ROWS = {0: (0, 10), 1: (10, 21), 2: (21, 32), 3: (32, 42), 4: (42, 48)}
            # stage-D row-blocks emitted in phase p (feat dependency permitting)
            D_IN_PHASE = {3: [0], 4: [1, 2]}

            for p in range(1, 5):
                ib = p - 1
                # denominator ones-matmuls for ib interleave with ib_p's
                # energy pairs on the PE (pair n+2 WAR-waits exp n: spacing
                # comes from the AV-T blocks / stage-D convs between pairs).
                emit_epair(p, 0)
                emit_epair(p, 1)
                emit_denom(ib)
                emit_epair(p, 2)
                emit_epair(p, 3)
                pav0 = emit_avt_cc(ib, 0)
                emit_epair(p, 4)
                emit_epair(p, 5)
                pav1 = emit_avt_cc(ib, 1)
                emit_epair(p, 6)
                emit_norm(ib, 0, pav0)
                emit_norm(ib, 1, pav1)
                emit_fpad_rows(*FPAD_ROWS[ib])
                d_list = D_IN_PHASE.get(p, [])
                if d_list:
                    emit_d_rb(d_list[0], between=[
                        lambda: emit_epair(p, 7), lambda: emit_epair(p, 8),
                    ])
                    for rb in d_list[1:]:
                        emit_d_rb(rb)
                else:
                    emit_epair(p, 7)
                    emit_epair(p, 8)

            # tail: stage-D row-block 3 first (covers the ib4 chunk-sum DVE
            # latency AND the denominator broadcast chain), then AV-T of the
            # last i-block — its normalize can then run immediately.
            h0, h1 = D_RB[3]
            o2 = outp.tile([128, 2, 480], BF16, tag="o")
            ps = emit_d_conv(h0, h1, 0)
            emit_d_act(ps, o2, h0, h1, 0)
            emit_denom(4)
            ps = emit_d_conv(h0, h1, 1)
            emit_d_act(ps, o2, h0, h1, 1)
            emit_d_out(o2, h0, h1)
            pav0 = emit_avt_cc(4, 0)
            emit_norm(4, 0, pav0)
            pav1 = emit_avt_cc(4, 1)
            emit_norm(4, 1, pav1)
            emit_fpad_rows(*FPAD_ROWS[4])
            emit_d_rb(4)
            emit_d_rb(5)

        if loop_reps:
            with tc.For_i(0, loop_reps, 1, hint_engines=tuple(nc.engines)):
                emit_body()
        else:
            for _ in range(sim_bodies):
                emit_body()

    nc.finalize()
    return nc


def get_nc():
    if "nc" not in _NC_CACHE:
        _NC_CACHE["nc"] = build_nc()
    return _NC_CACHE["nc"]


def make_in_maps(
    x, w_pre, b_pre, bn1_g, bn1_b, bn1_m, bn1_v,
    w_q, b_q, w_k, b_k, w_v, b_v,
    w_f, b_f, bn2_g, bn2_b, bn2_m, bn2_v, gamma,
):
    import ml_dtypes

    f = np.float32
    # host-pad x to [B, CIN, 50, 50] with zero borders -> line-rate DMA
    x = np.ascontiguousarray(x, f).reshape(B, CIN, H, W)
    xp = np.zeros((B, CIN, HP, WP), f)
    xp[:, :, 1 : 1 + H, 1 : 1 + W] = x
    xp = xp.reshape(B, CIN, NP2)

    # w_pre [256,512,3,3] -> [ci_part, m*36 + kc*9 + ty*3+tx, co_part]
    wp = np.ascontiguousarray(w_pre, f).reshape(2, 128, 4, 128, 3, 3)
    wpre = np.ascontiguousarray(wp.transpose(3, 0, 2, 4, 5, 1).reshape(128, 72, 128))
    # w_f [256,256,3,3] -> [ci_part, m*18 + kc*9 + ty*3+tx, co_part] (bf16)
    wf_ = np.ascontiguousarray(w_f, f).reshape(2, 128, 2, 128, 3, 3)
    wf = np.ascontiguousarray(
        wf_.transpose(3, 0, 2, 4, 5, 1).reshape(128, 36, 128)
    ).astype(ml_dtypes.bfloat16)
    # q/k weights duplicated across both output halves -> [ci_part, kc, co']
    wq2 = np.tile(np.asarray(w_q, f).reshape(CQK, COUT), (2, 1))
    wk2 = np.tile(np.asarray(w_k, f).reshape(CQK, COUT), (2, 1))
    wqk_st = np.concatenate([wq2, wk2], axis=0).reshape(256, 2, 128)
    wqk = np.ascontiguousarray(wqk_st.transpose(2, 1, 0))
    # gamma-scaled v weights [256,256] -> [ci_part, kc, co]
    g = f(np.asarray(gamma).reshape(-1)[0])
    wv_t = (np.asarray(w_v, f).reshape(COUT, COUT) * g).reshape(COUT, 2, 128)
    wv = np.ascontiguousarray(wv_t.transpose(2, 1, 0))

    s1 = np.asarray(bn1_g, f) / np.sqrt(np.asarray(bn1_v, f) + EPS)
    t1 = np.asarray(bn1_b, f) - np.asarray(bn1_m, f) * s1 + s1 * np.asarray(b_pre, f)
    s2 = np.asarray(bn2_g, f) / np.sqrt(np.asarray(bn2_v, f) + EPS)
    t2 = np.asarray(bn2_b, f) - np.asarray(bn2_m, f) * s2 + s2 * np.asarray(b_f, f)
    gbv = np.asarray(b_v, f) * g
    consts = np.zeros((128, 12), f)
    for m in range(2):
        consts[:, m] = s1[m * 128 : (m + 1) * 128]
        consts[:, 2 + m] = t1[m * 128 : (m + 1) * 128]
        consts[:, 4 + m] = s2[m * 128 : (m + 1) * 128]
        consts[:, 6 + m] = t2[m * 128 : (m + 1) * 128]
        consts[:, 10 + m] = gbv[m * 128 : (m + 1) * 128]
    consts[:, 8] = np.tile(np.asarray(b_q, f), 2)
    consts[:, 9] = np.tile(np.asarray(b_k, f), 2)

    bf = ml_dtypes.bfloat16
    shared = {
        "wpre": wpre.astype(bf), "wqk": wqk.astype(bf), "wv": wv.astype(bf),
        "wf": wf, "consts": consts,
    }
    xpb = xp.astype(bf)
    return [dict(shared, x=np.ascontiguousarray(xpb[b])) for b in range(B)]


def kernel(**inputs) -> np.ndarray:
    nc = get_nc()
    in_maps = make_in_maps(**inputs)
    res = run_bass_kernel_spmd(nc, in_maps, core_ids=list(range(B)))
    return np.stack(
        [res.results[b]["out"].reshape(COUT, H, W) for b in range(B)]
    ).astype(np.float32)


if __name__ == "__main__":
    nc = build_nc()
    print("build + finalize OK")



# revision 24
# speedup vs baseline: 1.0777x; 1.0777x over previous
"""Trainium2 Bass kernel for nn_AttentionLayer (pre-conv + BN/ReLU, QK^T
softmax attention, V aggregation, residual, final conv + BN/ReLU).

Data-parallel over batch: 8 samples -> 8 NeuronCores, zero collectives.
Everything streams bf16 into fp32 psum; output DMA'd as bf16.

Software-pipelined single-stream schedule (one For_i body = one sample):
  - Stage A (3x3 conv 512->256, 36 shifted matmuls/row-block) interleaves
    the 1x1 q/k convs, the vT chunks, and i-block-0's energy pairs as soon
    as their a1/q/k columns complete, so the ScalarE exp chain starts ~60us
    before stage A ends. Head DMA pieces alternate between the sync/scalar
    rings in exact first-use order (shared serial HWDGE + DMA pool).
  - Stage C phases: energy pairs of i-block k interleave with the
    transposed-AV matmuls of i-block k-1 (ut double-buffered by phase
    parity), so the PE never idles on the ScalarE exp chain.
  - Transposed AV: out[c,i] = sum_j vT[j,c]*ut[j,i] -- the attention output
    lands directly in channel-major layout (no per-isub PE transposes).
    The softmax denominator comes from DVE chunk-sums of ut, a
    column-of-ones matmul per 128-i chunk, tiny PE transposes, and one
    gpsimd partition_broadcast; the normalize + gamma*b_v + residual are
    two DVE ops per channel half.
  - Stage D (final 3x3 conv) row-blocks interleave into phases 3+ and the
    tail; the last row-block is 2 rows so the final act->dma->barrier
    chain after the last matmul is short.
"""
import sys

sys.path.insert(0, "/opt/trn_rl_repo")

import numpy as np

import concourse.bass as bass  # noqa: F401
import concourse.mybir as mybir
import concourse.tile as tile
from concourse import bacc
from concourse.bass_utils import run_bass_kernel_spmd
from concourse.masks import make_identity

F32 = mybir.dt.float32
BF16 = mybir.dt.bfloat16

B, CIN, COUT, H, W = 8, 512, 256, 48, 48
HP, WP = H + 2, W + 2  # padded feature map
NP2 = HP * WP  # 2500
N = H * W  # 2304
CQK = 64
NJ = N // 128  # 18
ROWBLOCKS = [(0, 10), (10, 20), (20, 30), (30, 40), (40, 48)]
IBLOCKS = [(0, 512), (512, 1024), (1024, 1536), (1536, 2048), (2048, 2304)]
EXP_SHIFT = -30.0  # exp(E + shift): cancels in softmax, guards overflow
EPS = 1e-5

_NC_CACHE = {}

ADD = mybir.AluOpType.add


def build_nc(loop_reps=None, sim_bodies=1, ablate=()):
    """loop_reps: wrap the body in a device-side For_i loop (timing builds).
    sim_bodies: without loop_reps, emit the body N times back-to-back
    (steady-state analysis under the timeline simulator).
    ablate: timing-diagnostic switches ("empty", "no_dma", "no_a", "no_attn",
    "no_d"); default () = the real kernel."""
    nc = bacc.Bacc("TRN2")

    x_d = nc.declare_dram_parameter("x", [CIN, NP2], BF16, isOutput=False)
    wpre_d = nc.declare_dram_parameter("wpre", [128, 72, 128], BF16, isOutput=False)
    wqk_d = nc.declare_dram_parameter("wqk", [128, 2, 256], BF16, isOutput=False)
    wv_d = nc.declare_dram_parameter("wv", [128, 2, 256], BF16, isOutput=False)
    wf_d = nc.declare_dram_parameter("wf", [128, 36, 128], BF16, isOutput=False)
    const_d = nc.declare_dram_parameter("consts", [128, 12], F32, isOutput=False)
    out_d = nc.declare_dram_parameter("out", [COUT, N], BF16, isOutput=True)

    RELU = mybir.ActivationFunctionType.Relu
    IDENT = mybir.ActivationFunctionType.Identity
    EXP = mybir.ActivationFunctionType.Exp

    with (
        tile.TileContext(nc) as tc,
        tc.tile_pool(name="consts", bufs=1) as consts,
        tc.tile_pool(name="data", bufs=1) as data,
        tc.tile_pool(name="attp", bufs=6) as attp,
        tc.tile_pool(name="outp", bufs=4) as outp,
        tc.tile_pool(name="acc", bufs=4, space="PSUM") as acc,
        tc.tile_pool(name="epool", bufs=2, space="PSUM") as epool,
    ):
        x_sb = data.tile([128, 4, HP, WP], BF16, tag="x")
        wpre_sb = consts.tile([128, 72, 128], BF16, tag="wpre")
        const_sb = consts.tile([128, 12], F32, tag="const")
        wqk_sb = consts.tile([128, 2, 256], BF16, tag="wqk")
        wv_sb = consts.tile([128, 2, 256], BF16, tag="wv")
        wf_sb = consts.tile([128, 36, 128], BF16, tag="wf")
        a1_sb = data.tile([128, 2, N], BF16, tag="a1")
        # q/k duplicated across both partition halves: enables row-packed
        # K=64 energy matmuls (two j-chunks concurrently in the PE array)
        q_sb = data.tile([128, N], BF16, tag="q")
        k_sb = data.tile([128, N], BF16, tag="k")
        vt_sb = data.tile([128, NJ, 256], BF16, tag="vt")
        # ut double-buffered by i-block parity: exp of i-block k writes
        # parity k%2 while the AV of i-block k-1 reads parity (k-1)%2.
        ut_sb = data.tile([128, 2, NJ, 512], BF16, tag="ut")
        feat_sb = data.tile([128, 2, N], BF16, tag="feat")
        fpad_sb = data.tile([128, 2, HP, WP], BF16, tag="fpad")
        ebias_sb = consts.tile([128, 1], F32, tag="ebias")
        ident_sb = consts.tile([128, 128], BF16, tag="ident")
        onec_sb = consts.tile([128, 1], BF16, tag="onec")
        # softmax-denominator scratch: per-i-block chunk sums of exp values
        # and the row-broadcast reciprocal used to normalize the AV output
        s_sb = data.tile([128, 512], BF16, tag="s")
        recb_sb = data.tile([128, 4, 128], BF16, tag="recb")

        def emit_loads():
            """Head DMAs. The HWDGE descriptor unit round-robins the sync and
            scalar rings and all transfers serialize on a shared DMA pool, so
            pieces alternate between rings in exact first-use order: wpre[0:3]
            | x kc0 head | wpre[3:9] | wpre kc1 | x kc1 head | ... so conv
            row-block 0 starts ~3us in."""

            def dma_x_head(ring, kc):
                ring.dma_start(
                    out=x_sb[:, kc, 0:13, :].rearrange("p h w -> p (h w)"),
                    in_=x_d[kc * 128 : (kc + 1) * 128, 0 : 13 * WP],
                )

            def dma_wpre(ring, s0, s1):
                ring.dma_start(out=wpre_sb[:, s0:s1, :], in_=wpre_d[:, s0:s1, :])

            def dma_x_band(ring, r0, r1):
                # rows r0:r1 of all 4 kc chunks (JIT bands for conv row-blocks)
                ring.dma_start(
                    out=x_sb[:, :, r0:r1, :].rearrange("p c h w -> p c (h w)"),
                    in_=x_d.rearrange("(c p) n -> p c n", c=4)[
                        :, :, r0 * WP : r1 * WP
                    ],
                )

            dma_wpre(nc.sync, 0, 3)
            dma_x_head(nc.sync, 0)
            dma_wpre(nc.sync, 3, 9)
            dma_wpre(nc.sync, 9, 18)
            dma_x_head(nc.sync, 1)
            dma_wpre(nc.sync, 18, 27)
            dma_x_head(nc.sync, 2)
            dma_wpre(nc.sync, 27, 36)
            dma_x_head(nc.sync, 3)
            dma_wpre(nc.sync, 36, 45)
            dma_wpre(nc.sync, 45, 54)
            dma_wpre(nc.sync, 54, 63)
            dma_wpre(nc.sync, 63, 72)
            nc.gpsimd.dma_start(out=const_sb[:], in_=const_d[:])
            # bulk of x (rows 13..49, all kc chunks) in one piece
            nc.sync.dma_start(
                out=x_sb[:, :, 13:, :].rearrange("p c h w -> p c (h w)"),
                in_=x_d.rearrange("(c p) n -> p c n", c=4)[:, :, 13 * WP :],
            )
            nc.gpsimd.dma_start(out=wqk_sb[:], in_=wqk_d[:])
            nc.gpsimd.dma_start(out=wv_sb[:], in_=wv_d[:])
            nc.gpsimd.dma_start(out=wf_sb[:], in_=wf_d[:])

        # ---- one-time setup: constants that no iteration clobbers
        nc.vector.memset(ebias_sb[:], EXP_SHIFT)
        nc.vector.memset(onec_sb[:], 1.0)
        make_identity(nc, ident_sb[:])
        # fpad borders stay zero; row copies only touch the interior
        nc.vector.memset(fpad_sb[:, :, 0:1, :], 0.0)
        nc.vector.memset(fpad_sb[:, :, HP - 1 : HP, :], 0.0)
        nc.vector.memset(fpad_sb[:, :, 1 : 1 + H, 0:1], 0.0)
        nc.vector.memset(fpad_sb[:, :, 1 : 1 + H, WP - 1 : WP], 0.0)

        def emit_body():
            if "empty" in ablate:
                nc.vector.memset(recb_sb[:, 0:1, 0:4], 0.0)
                return
            if "only_dma" in ablate:
                emit_loads()
                return
            if "no_dma" not in ablate:
                emit_loads()
            else:
                # satisfy the tile allocator: every read tile needs a writer
                nc.vector.memset(x_sb[:, :, 0:1, 0:4], 0.0)
                nc.vector.memset(wpre_sb[:, 0:1, 0:4], 0.0)
                nc.vector.memset(wqk_sb[:, 0:1, 0:4], 0.0)
                nc.vector.memset(wv_sb[:, 0:1, 0:4], 0.0)
                nc.vector.memset(wf_sb[:, 0:1, 0:4], 0.0)
                nc.vector.memset(const_sb[:, 0:4], 0.0)
            if "no_a" in ablate:
                nc.vector.memset(a1_sb[:, :, 0:2], 0.0)
            if "no_attn" in ablate:
                nc.vector.memset(feat_sb[:, :, 0:2], 0.0)
            if "no_exp" in ablate or "no_energy" in ablate:
                nc.vector.memset(ut_sb[:, :, 0:1, 0:2], 0.01)
            if "no_den" in ablate:
                nc.vector.memset(recb_sb[:, :, 0:2], 1.0)

            # ---------------- emit helpers ----------------

            def conv3x3(ps, w_sb, slot_of, src4, kcs, h0, h1):
                taps = [
                    (kc, ty, tx)
                    for kc in range(kcs)
                    for ty in range(3)
                    for tx in range(3)
                ]
                for idx, (kc, ty, tx) in enumerate(taps):
                    nc.tensor.matmul(
                        ps[:, : (h1 - h0) * W],
                        lhsT=w_sb[:, slot_of(kc, ty, tx), :],
                        rhs=src4[:, kc, ty + h0 : ty + h1, tx : tx + W],
                        start=(idx == 0),
                        stop=(idx == len(taps) - 1),
                    )

            def emit_a_conv(h0, h1, m):
                wblk = (h1 - h0) * W
                ps = acc.tile([128, 512], F32, tag="acc")
                conv3x3(
                    ps, wpre_sb,
                    lambda kc, ty, tx, m=m: m * 36 + kc * 9 + ty * 3 + tx,
                    x_sb, 4, h0, h1,
                )
                nc.scalar.activation(
                    a1_sb[:, m, h0 * W : h1 * W], ps[:, :wblk], RELU,
                    scale=const_sb[:, m : m + 1],
                    bias=const_sb[:, 2 + m : 3 + m],
                )

            def emit_k(h0, h1):
                i0, i1 = h0 * W, h1 * W
                wi = i1 - i0
                psk = acc.tile([128, 512], F32, tag="acc")
                for kc in range(2):
                    nc.tensor.matmul(
                        psk[:, :wi],
                        lhsT=wqk_sb[:, kc, 128:256],
                        rhs=a1_sb[:, kc, i0:i1],
                        start=(kc == 0), stop=(kc == 1),
                    )
                nc.scalar.activation(
                    k_sb[:, i0:i1], psk[:, :wi], IDENT, bias=const_sb[:, 9:10]
                )

            def emit_q(h0, h1):
                i0, i1 = h0 * W, h1 * W
                wi = i1 - i0
                psq = acc.tile([128, 512], F32, tag="acc")
                for kc in range(2):
                    nc.tensor.matmul(
                        psq[:, :wi],
                        lhsT=wqk_sb[:, kc, 0:128],
                        rhs=a1_sb[:, kc, i0:i1],
                        start=(kc == 0), stop=(kc == 1),
                    )
                nc.scalar.activation(
                    q_sb[:, i0:i1], psq[:, :wi], IDENT, bias=const_sb[:, 8:9]
                )

            def emit_vt(j):
                psv = acc.tile([128, 512], F32, tag="acc")
                for kc in range(2):
                    nc.tensor.matmul(
                        psv[:, :256],
                        lhsT=a1_sb[:, kc, j * 128 : (j + 1) * 128],
                        rhs=wv_sb[:, kc, :],
                        start=(kc == 0), stop=(kc == 1),
                    )
                nc.vector.tensor_copy(vt_sb[:, j, 0:256], psv[:, :256])

            E_PAIR = 2

            def emit_epair(ib, jj):
                i0, i1 = IBLOCKS[ib]
                wi = i1 - i0
                par = ib % 2
                if "no_energy" in ablate:
                    return
                pse = epool.tile([128, E_PAIR, 512], F32, tag="e")
                for hh in range(E_PAIR):
                    j = E_PAIR * jj + hh
                    p0 = (hh % 2) * CQK  # alternate array row-halves
                    nc.tensor.matmul(
                        pse[:, hh, :wi],
                        lhsT=k_sb[p0 : p0 + CQK, j * 128 : (j + 1) * 128],
                        rhs=q_sb[p0 : p0 + CQK, i0:i1],
                        start=True, stop=True,
                    )
                if "no_exp" in ablate:
                    return
                nc.scalar.activation(
                    ut_sb[:, par, E_PAIR * jj : E_PAIR * (jj + 1), :wi],
                    pse[:, :, :wi], EXP,
                    bias=ebias_sb[:, 0:1],
                )

            # ---- transposed AV: out[c, i] = sum_j vt[j, c] * ut[j, i].
            # No PE transposes; the softmax denominator comes from DVE chunk
            # sums of ut + one column-of-ones matmul per 128-i chunk, and the
            # per-i reciprocal is row-broadcast via gpsimd partition_broadcast.

            def emit_avt_cc(ib, cc):
                i0, i1 = IBLOCKS[ib]
                wi = i1 - i0
                par = ib % 2
                pav = acc.tile([128, 512], F32, tag="acc")
                nj = 1 if "no_av" in ablate else NJ
                for j in range(nj):
                    nc.tensor.matmul(
                        pav[:, :wi],
                        lhsT=vt_sb[:, j, cc * 128 : (cc + 1) * 128],
                        rhs=ut_sb[:, par, j, 0:wi],
                        start=(j == 0), stop=(j == nj - 1),
                    )
                return pav

            def emit_chunk_sums(ib):
                # bf16 accumulation: ~1.7% worst-case on the denominator,
                # diluted by gamma=0.1 on the residual -> ~0.2% on the output
                if "no_den" in ablate:
                    return
                par = ib % 2
                wi = IBLOCKS[ib][1] - IBLOCKS[ib][0]
                with nc.allow_low_precision(reason="softmax denom, gamma-diluted"):
                    nc.vector.tensor_add(
                        s_sb[:, :wi], ut_sb[:, par, 0, 0:wi], ut_sb[:, par, 1, 0:wi]
                    )
                    for j in range(2, NJ):
                        nc.vector.tensor_add(
                            s_sb[:, :wi], s_sb[:, :wi], ut_sb[:, par, j, 0:wi]
                        )

            def emit_denom(ib):
                if "no_den" in ablate:
                    return
                wi = IBLOCKS[ib][1] - IBLOCKS[ib][0]
                nch = wi // 128
                dnt = acc.tile([128, 512], F32, tag="acc")
                for ic in range(nch):
                    nc.tensor.matmul(
                        dnt[:, ic : ic + 1],
                        lhsT=s_sb[:, ic * 128 : (ic + 1) * 128],
                        rhs=onec_sb[:, 0:1],
                        start=True, stop=True,
                    )
                rec = attp.tile([128, 4], BF16, tag="rec")
                with nc.allow_low_precision(reason="softmax 1/denom, gamma-diluted"):
                    nc.vector.reciprocal(rec[:, 0:nch], dnt[:, 0:nch])
                # i: partitions -> free. One tiny PE transpose per 128-chunk,
                # all landing on partition 0 of a bf16-viewed region of the
                # same bank; gpsimd can't read PSUM so bounce through SBUF.
                dntb = dnt.bitcast(BF16)  # [128, 1024] view
                for ic in range(nch):
                    nc.tensor.transpose(
                        dntb[0:1, 128 + ic * 128 : 256 + ic * 128],
                        rec[:, ic : ic + 1],
                        ident_sb[:],
                    )
                rrow = attp.tile([1, 512], BF16, tag="rrow")
                nc.vector.tensor_copy(
                    rrow[0:1, 0 : nch * 128], dntb[0:1, 128 : 128 + nch * 128]
                )
                nc.gpsimd.partition_broadcast(
                    recb_sb[:].rearrange("p a b -> p (a b)")[:, 0 : nch * 128],
                    rrow[0:1, 0 : nch * 128],
                )

            def emit_norm(ib, cc, pav):
                i0, i1 = IBLOCKS[ib]
                wi = i1 - i0
                u1 = attp.tile([128, 512], BF16, tag="att")
                nc.vector.tensor_mul(
                    u1[:, :wi], pav[:, :wi],
                    recb_sb[:].rearrange("p a b -> p (a b)")[:, :wi],
                )
                # feat = (att_norm + gamma*b_v) + a1
                nc.vector.scalar_tensor_tensor(
                    feat_sb[:, cc, i0:i1],
                    u1[:, :wi],
                    const_sb[:, 10 + cc : 11 + cc],
                    a1_sb[:, cc, i0:i1],
                    op0=ADD, op1=ADD,
                )

            def emit_fpad_rows(r0, r1):
                for cc in range(2):
                    nc.vector.tensor_copy(
                        fpad_sb[:, cc, 1 + r0 : 1 + r1, 1 : 1 + W],
                        feat_sb[:, cc, r0 * W : r1 * W].rearrange(
                            "p (h w) -> p h w", w=W
                        ),
                    )

            def emit_d_conv(h0, h1, m):
                ps = acc.tile([128, 512], F32, tag="acc")
                conv3x3(
                    ps, wf_sb,
                    lambda kc, ty, tx, m=m: m * 18 + kc * 9 + ty * 3 + tx,
                    fpad_sb, 2, h0, h1,
                )
                return ps

            def emit_d_act(ps, o2, h0, h1, m):
                wblk = (h1 - h0) * W
                nc.scalar.activation(
                    o2[:, m, :wblk], ps[:, :wblk], RELU,
                    scale=const_sb[:, 4 + m : 5 + m],
                    bias=const_sb[:, 6 + m : 7 + m],
                )

            def emit_d_out(o2, h0, h1, ring=None):
                wblk = (h1 - h0) * W
                (ring or nc.scalar).dma_start(
                    out=out_d.rearrange("(m p) n -> p m n", m=2)[
                        :, :, h0 * W : h1 * W
                    ],
                    in_=o2[:, :, :wblk],
                )

            # stage-D row-blocks: last block kept tiny so the final
            # act->dma->barrier tail after the last matmul is short.
            D_RB = [(0, 10), (10, 20), (20, 30), (30, 40), (40, 46), (46, 48)]

            def emit_d_rb(rb, between=(), ring=None):
                """Emit one stage-D row-block; `between` is a list of thunks
                interleaved at the two conv-half boundaries (keeps exp ahead
                of relu on the ScalarE queue)."""
                h0, h1 = D_RB[rb]
                its = list(between)
                o2 = outp.tile([128, 2, 480], BF16, tag="o")
                for m in range(2):
                    ps = emit_d_conv(h0, h1, m)
                    if its:
                        its.pop(0)()
                    emit_d_act(ps, o2, h0, h1, m)
                for t in its:
                    t()
                emit_d_out(o2, h0, h1, ring)

            # ---------------- schedule ----------------

            if ablate:
                # diagnostic schedules only (default path below is untouched)
                if "no_a" not in ablate:
                    for r, (h0, h1) in enumerate(ROWBLOCKS):
                        emit_a_conv(h0, h1, 0)
                        emit_a_conv(h0, h1, 1)
                if "no_attn" not in ablate:
                    for rb in range(5):
                        emit_k(*ROWBLOCKS[rb])
                        emit_q(*ROWBLOCKS[rb])
                    for j in range(NJ):
                        emit_vt(j)
                    for p in range(5):
                        ib = p
                        for jj in range(9):
                            emit_epair(ib, jj)
                        emit_chunk_sums(ib)
                        pav0 = emit_avt_cc(ib, 0)
                        pav1 = emit_avt_cc(ib, 1)
                        emit_denom(ib)
                        emit_norm(ib, 0, pav0)
                        emit_norm(ib, 1, pav1)
                if "no_d" not in ablate:
                    emit_fpad_rows(0, 48)
                    for rb in range(len(D_RB)):
                        emit_d_rb(rb)
                else:
                    nc.scalar.dma_start(
                        out=out_d.rearrange("(m p) n -> p m n", m=2),
                        in_=feat_sb[:, :, :],
                    )
                return

            # Stage A/B: conv row-blocks; row-block r's vT/q/k/i-block-0
            # energy pairs are deferred until after conv(r+1, m0) so their
            # psum-bank WARs and a1-activation dependencies are long settled.
            A_TAIL = {
                0: [],
                1: [("e", (0, 3))],
                2: [("e", (3, 5))],
                3: [("e", (5, 7))],
            }
            VT_OF_RB = {0: (0, 3), 1: (3, 7), 2: (7, 11), 3: (11, 15), 4: (15, 18)}
            for r, (h0, h1) in enumerate(ROWBLOCKS):
                emit_a_conv(h0, h1, 0)
                emit_a_conv(h0, h1, 1)
                if r > 0:
                    pr = r - 1
                    ph0, ph1 = ROWBLOCKS[pr]
                    for j in range(*VT_OF_RB[pr]):
                        emit_vt(j)
                    emit_k(ph0, ph1)
                    emit_q(ph0, ph1)
                    for kind, (a, b) in A_TAIL[pr]:
                        for idx in range(a, b):
                            emit_epair(0, idx)
            # row-block 4 tail: k/q/vT acts first so the two big i-block-0
            # exps don't block their psum drains on the ScalarE queue; the
            # energy pairs space the last vT copies.
            emit_k(*ROWBLOCKS[4])
            emit_q(*ROWBLOCKS[4])
            emit_vt(15)
            emit_epair(0, 7)
            emit_vt(16)
            emit_epair(0, 8)
            emit_vt(17)

            # fpad rows fully determined by each i-block's feat columns
            FPAD_ROWS = {0: (0, 10), 1: (10, 21), 2: (21, 32), 3: (32, 42), 4: (42, 48)}
            # stage-D row-blocks emitted in phase p (feat dependency permitting)
            D_IN_PHASE = {3: [0], 4: [1, 2]}

            for p in range(1, 5):
                ib = p - 1
                # denominator chunk sums for ib run on the DVE while the PE
                # streams the AV-T matmuls, both interleaved with ib_p's
                # energy pairs (pair n+2 WAR-waits exp n: spacing comes from
                # the AV-T blocks / stage-D convs between pairs).
                emit_chunk_sums(ib)
                emit_epair(p, 0)
                emit_epair(p, 1)
                pav0 = emit_avt_cc(ib, 0)
                emit_epair(p, 2)
                emit_epair(p, 3)
                pav1 = emit_avt_cc(ib, 1)
                emit_epair(p, 4)
                emit_epair(p, 5)
                emit_denom(ib)
                emit_epair(p, 6)
                emit_norm(ib, 0, pav0)
                emit_norm(ib, 1, pav1)
                emit_fpad_rows(*FPAD_ROWS[ib])
                d_list = D_IN_PHASE.get(p, [])
                if d_list:
                    emit_d_rb(d_list[0], between=[
                        lambda: emit_epair(p, 7), lambda: emit_epair(p, 8),
                    ])
                    for rb in d_list[1:]:
                        emit_d_rb(rb)
                else:
                    emit_epair(p, 7)
                    emit_epair(p, 8)

            # tail: stage-D row-block 3 first (covers the ib4 chunk-sum DVE
            # latency AND the denominator broadcast chain), then AV-T of the
            # last i-block — its normalize can then run immediately.
            emit_chunk_sums(4)
            h0, h1 = D_RB[3]
            o2 = outp.tile([128, 2, 480], BF16, tag="o")
            ps = emit_d_conv(h0, h1, 0)
            emit_d_act(ps, o2, h0, h1, 0)
            emit_denom(4)
            ps = emit_d_conv(h0, h1, 1)
            emit_d_act(ps, o2, h0, h1, 1)
            emit_d_out(o2, h0, h1)
            pav0 = emit_avt_cc(4, 0)
            emit_norm(4, 0, pav0)
            pav1 = emit_avt_cc(4, 1)
            emit_norm(4, 1, pav1)
            emit_fpad_rows(*FPAD_ROWS[4])
            emit_d_rb(4)
            emit_d_rb(5)

        if loop_reps:
            with tc.For_i(0, loop_reps, 1, hint_engines=tuple(nc.engines)):
                emit_body()
        else:
            for _ in range(sim_bodies):
                emit_body()

    nc.finalize()
    return nc


def get_nc():
    if "nc" not in _NC_CACHE:
        _NC_CACHE["nc"] = build_nc()
    return _NC_CACHE["nc"]


def make_in_maps(
    x, w_pre, b_pre, bn1_g, bn1_b, bn1_m, bn1_v,
    w_q, b_q, w_k, b_k, w_v, b_v,
    w_f, b_f, bn2_g, bn2_b, bn2_m, bn2_v, gamma,
):
    import ml_dtypes

    f = np.float32
    # host-pad x to [B, CIN, 50, 50] with zero borders -> line-rate DMA
    x = np.ascontiguousarray(x, f).reshape(B, CIN, H, W)
    xp = np.zeros((B, CIN, HP, WP), f)
    xp[:, :, 1 : 1 + H, 1 : 1 + W] = x
    xp = xp.reshape(B, CIN, NP2)

    # w_pre [256,512,3,3] -> [ci_part, m*36 + kc*9 + ty*3+tx, co_part]
    wp = np.ascontiguousarray(w_pre, f).reshape(2, 128, 4, 128, 3, 3)
    wpre = np.ascontiguousarray(wp.transpose(3, 0, 2, 4, 5, 1).reshape(128, 72, 128))
    # w_f [256,256,3,3] -> [ci_part, m*18 + kc*9 + ty*3+tx, co_part] (bf16)
    wf_ = np.ascontiguousarray(w_f, f).reshape(2, 128, 2, 128, 3, 3)
    wf = np.ascontiguousarray(
        wf_.transpose(3, 0, 2, 4, 5, 1).reshape(128, 36, 128)
    ).astype(ml_dtypes.bfloat16)
    # q/k weights duplicated across both output halves -> [ci_part, kc, co']
    wq2 = np.tile(np.asarray(w_q, f).reshape(CQK, COUT), (2, 1))
    wk2 = np.tile(np.asarray(w_k, f).reshape(CQK, COUT), (2, 1))
    wqk_st = np.concatenate([wq2, wk2], axis=0).reshape(256, 2, 128)
    wqk = np.ascontiguousarray(wqk_st.transpose(2, 1, 0))
    # gamma-scaled v weights [256,256] -> [ci_part, kc, co]
    g = f(np.asarray(gamma).reshape(-1)[0])
    wv_t = (np.asarray(w_v, f).reshape(COUT, COUT) * g).reshape(COUT, 2, 128)
    wv = np.ascontiguousarray(wv_t.transpose(2, 1, 0))

    s1 = np.asarray(bn1_g, f) / np.sqrt(np.asarray(bn1_v, f) + EPS)
    t1 = np.asarray(bn1_b, f) - np.asarray(bn1_m, f) * s1 + s1 * np.asarray(b_pre, f)
    s2 = np.asarray(bn2_g, f) / np.sqrt(np.asarray(bn2_v, f) + EPS)
    t2 = np.asarray(bn2_b, f) - np.asarray(bn2_m, f) * s2 + s2 * np.asarray(b_f, f)
    gbv = np.asarray(b_v, f) * g
    consts = np.zeros((128, 12), f)
    for m in range(2):
        consts[:, m] = s1[m * 128 : (m + 1) * 128]
        consts[:, 2 + m] = t1[m * 128 : (m + 1) * 128]
        consts[:, 4 + m] = s2[m * 128 : (m + 1) * 128]
        consts[:, 6 + m] = t2[m * 128 : (m + 1) * 128]
        consts[:, 10 + m] = gbv[m * 128 : (m + 1) * 128]
    consts[:, 8] = np.tile(np.asarray(b_q, f), 2)
    consts[:, 9] = np.tile(np.asarray(b_k, f), 2)

    bf = ml_dtypes.bfloat16
    shared = {
        "wpre": wpre.astype(bf), "wqk": wqk.astype(bf), "wv": wv.astype(bf),
        "wf": wf, "consts": consts,
    }
    xpb = xp.astype(bf)
    return [dict(shared, x=np.ascontiguousarray(xpb[b])) for b in range(B)]


def kernel(**inputs) -> np.ndarray:
    nc = get_nc()
    in_maps = make_in_maps(**inputs)
    res = run_bass_kernel_spmd(nc, in_maps, core_ids=list(range(B)))
    return np.stack(
        [res.results[b]["out"].reshape(COUT, H, W) for b in range(B)]
    ).astype(np.float32)


if __name__ == "__main__":
    nc = build_nc()
    print("build + finalize OK")

